# revision 5
# baseline (speedup 1.0000x reference)
"""Distributed FlashRotarySelfAttention kernel for 8 TRN2 NeuronCores.

Reference computation (per nn_FlashRotarySelfAttention):
  qkv = x @ Wqkv;  k, q, v = split(qkv, 3)  [k first!]
  k, q = rope(k), rope(q)
  out = causal_softmax(q k^T / sqrt(Dh)) @ v
  return out @ Wproj

Sharding: tensor-parallel over heads. Core i owns heads {2i, 2i+1}:
  - column-parallel Wqkv (k|q|v columns of its 2 heads)
  - attention fully local per (batch, head)
  - per-(batch, q-chunk) AllGather of attention outputs (transposed,
    c-major), pipelined against attention compute
  - column-parallel Wproj: each core computes 256 output channels
Host concatenates + transposes the per-core outputs.

All matmuls run in bf16 with fp32 PSUM accumulation. Softmax skips the
max-subtraction (scores are O(10) here, exp is safe in fp32); the
denominator comes from a ones-matmul that replicates it across all 128
partitions so the normalization is a plain aligned elementwise multiply.
"""

from contextlib import ExitStack

import numpy as np
import ml_dtypes

import concourse.bacc as bacc
import concourse.mybir as mybir
import concourse.tile as tile
from concourse.bass_utils import run_bass_kernel_spmd

# Problem shapes (hardcoded per contest rules).
B, S, C, H = 2, 2048, 2048, 16
Dh = C // H                      # 128
BS = B * S                       # 4096
N_CORES = 8
H_LOC = H // N_CORES             # 2 heads per core
W_LOC = 3 * H_LOC * Dh           # 768 local qkv columns
CO_LOC = C // N_CORES            # 256 output channels per core
ROPE_THETA = 10000.0
SCALE = float(Dh) ** -0.5

F32 = mybir.dt.float32
BF16 = mybir.dt.bfloat16

P = 128            # partitions
QCH = 512          # q-chunk (matmul free dim)
N_SC = BS // QCH   # 8 s-chunks over B*S
N_CC = C // P      # 16 contraction chunks
N_QC = S // QCH    # 4 q-chunks per batch
N_KT = S // P      # 16 k-tiles per batch


def _host_constants():
    """Input-independent tables computed on host (compile-time constants)."""
    half = Dh // 2
    inv_freq = 1.0 / (ROPE_THETA ** (np.arange(0, half, dtype=np.float64) / half))
    ang = np.arange(S, dtype=np.float64)[None, :] * inv_freq[:, None]   # [64, S]
    cos_t = np.cos(ang).astype(np.float32)
    sin_t = np.sin(ang).astype(np.float32)
    # Causal 0/1 masks for diagonal score tiles, scoresT layout [k_local, q_local].
    # Tile j (k-tile index j within the q-chunk): keep iff q_local >= 128*j + k_local.
    kk = np.arange(P)[:, None]
    qq = np.arange(QCH)[None, :]
    masks = np.stack(
        [(qq >= P * j + kk) for j in range(4)], axis=0
    ).astype(ml_dtypes.bfloat16)                                        # [4, 128, 512]
    ident = np.eye(P, dtype=ml_dtypes.bfloat16)
    ones = np.ones((P, P), dtype=ml_dtypes.bfloat16)
    return cos_t, sin_t, masks, ident, ones


def build_nc():
    nc = bacc.Bacc(None, num_devices=N_CORES)

    x_in = nc.declare_dram_parameter("x", [BS, C], F32, isOutput=False)
    wqkv_in = nc.declare_dram_parameter("wqkv", [C, W_LOC], F32, isOutput=False)
    wproj_in = nc.declare_dram_parameter("wproj", [C, CO_LOC], F32, isOutput=False)
    cos_in = nc.declare_dram_parameter("cos_t", [Dh // 2, S], F32, isOutput=False)
    sin_in = nc.declare_dram_parameter("sin_t", [Dh // 2, S], F32, isOutput=False)
    masks_in = nc.declare_dram_parameter("masks", [4, P, QCH], BF16, isOutput=False)
    ident_in = nc.declare_dram_parameter("ident", [P, P], BF16, isOutput=False)
    ones_in = nc.declare_dram_parameter("ones", [P, P], BF16, isOutput=False)
    out_ext = nc.declare_dram_parameter("outT", [CO_LOC, BS], F32, isOutput=True)

    with tile.TileContext(nc) as tc, ExitStack() as ctx:
        consts = ctx.enter_context(tc.tile_pool(name="consts", bufs=1))
        qkvp = ctx.enter_context(tc.tile_pool(name="qkvp", bufs=1))
        xt_pool = ctx.enter_context(tc.tile_pool(name="xt", bufs=2))
        rope_pool = ctx.enter_context(tc.tile_pool(name="rope", bufs=8))
        probs_pool = ctx.enter_context(tc.tile_pool(name="probs", bufs=4))
        vtmp_pool = ctx.enter_context(tc.tile_pool(name="vtmp", bufs=2))
        attn_pool = ctx.enter_context(tc.tile_pool(name="attn", bufs=2))
        gt_pool = ctx.enter_context(tc.tile_pool(name="gt", bufs=2))
        outp_pool = ctx.enter_context(tc.tile_pool(name="outp", bufs=2))
        dram = ctx.enter_context(tc.tile_pool(name="dram", bufs=1, space="DRAM"))
        mmps = ctx.enter_context(tc.tile_pool(name="mmps", bufs=2, space="PSUM"))
        ops_pool = ctx.enter_context(tc.tile_pool(name="ops", bufs=2, space="PSUM"))
        dps_pool = ctx.enter_context(tc.tile_pool(name="dps", bufs=2, space="PSUM"))
        vtps = ctx.enter_context(tc.tile_pool(name="vtps", bufs=2, space="PSUM"))

        # ---- Phase 1a: first x chunk cast starts immediately -------------
        x_chunks = [dram.tile([QCH, C], BF16, name=f"xch{j}") for j in range(N_SC)]
        nc.gpsimd.dma_start(x_chunks[0][:], x_in[0:QCH, :])

        # ---- Phase 0: constants / weights to SBUF ------------------------
        wqkv_sb = consts.tile([P, N_CC, W_LOC], BF16)
        nc.gpsimd.dma_start(wqkv_sb[:], wqkv_in.rearrange("(o p) w -> p o w", p=P))
        wproj_sb = consts.tile([P, N_CC, CO_LOC], BF16)
        nc.gpsimd.dma_start(wproj_sb[:], wproj_in.rearrange("(o p) w -> p o w", p=P))
        cos_sb = consts.tile([Dh // 2, S], F32)
        nc.scalar.dma_start(cos_sb[:], cos_in[:])
        sin_sb = consts.tile([Dh // 2, S], F32)
        nc.scalar.dma_start(sin_sb[:], sin_in[:])
        masks_sb = consts.tile([P, 4, QCH], BF16)
        nc.scalar.dma_start(masks_sb[:], masks_in.rearrange("j p q -> p j q"))
        ident_sb = consts.tile([P, P], BF16)
        nc.scalar.dma_start(ident_sb[:], ident_in[:])
        ones_sb = consts.tile([P, P], BF16)
        nc.scalar.dma_start(ones_sb[:], ones_in[:])

        # Remaining x chunks
        for sc in range(1, N_SC):
            nc.gpsimd.dma_start(x_chunks[sc][:], x_in[sc * QCH:(sc + 1) * QCH, :])

        # Resident activations: d-major q/k, k-major v. bh = h_local*2 + b
        q_sb = qkvp.tile([P, 2 * H_LOC, S], BF16)
        k_sb = qkvp.tile([P, 2 * H_LOC, S], BF16)
        v_sb = qkvp.tile([P, 2 * H_LOC, N_KT, Dh], BF16)

        # ---- Phase 2: transpose-load x^T, QKV matmuls, RoPE --------------
        for sc in range(N_SC):
            b = sc // N_QC
            s0 = (sc % N_QC) * QCH       # position offset within batch
            cos_c = cos_sb[:, s0:s0 + QCH]
            sin_c = sin_sb[:, s0:s0 + QCH]
            # x^T tile [c_in(128, o), s(512)] via XBAR transpose
            xt = xt_pool.tile([P, N_CC, QCH], BF16)
            nc.sync.dma_start_transpose(xt[:], x_chunks[sc][:])

            for ct in range(6):
                ps = mmps.tile([P, QCH], F32, tag="mm")
                for cc in range(N_CC):
                    nc.tensor.matmul(
                        ps[:],
                        lhsT=wqkv_sb[:, cc, ct * P:(ct + 1) * P],
                        rhs=xt[:, cc, :],
                        start=(cc == 0),
                        stop=(cc == N_CC - 1),
                    )
                if ct < 4:
                    # k (ct 0,1) and q (ct 2,3): RoPE -> bf16 resident
                    hl = ct % 2
                    dst = k_sb if ct < 2 else q_sb
                    bh = hl * 2 + b
                    lo = ps[0:64, :]
                    hi = ps[64:128, :]
                    t1 = rope_pool.tile([64, QCH], F32, tag="rt")
                    t2 = rope_pool.tile([64, QCH], F32, tag="rt")
                    t3 = rope_pool.tile([64, QCH], F32, tag="rt")
                    t4 = rope_pool.tile([64, QCH], F32, tag="rt")
                    nc.any.tensor_tensor(t1[:], lo, cos_c, mybir.AluOpType.mult)
                    nc.any.tensor_tensor(t2[:], hi, sin_c, mybir.AluOpType.mult)
                    nc.any.tensor_tensor(
                        dst[0:64, bh, s0:s0 + QCH],
                        t1[:], t2[:], mybir.AluOpType.subtract,
                    )
                    nc.any.tensor_tensor(t3[:], hi, cos_c, mybir.AluOpType.mult)
                    nc.any.tensor_tensor(t4[:], lo, sin_c, mybir.AluOpType.mult)
                    nc.any.tensor_tensor(
                        dst[64:128, bh, s0:s0 + QCH],
                        t3[:], t4[:], mybir.AluOpType.add,
                    )
                else:
                    # v (ct 4,5): cast to bf16, PE-transpose to k-major
                    hl = ct - 4
                    bh = hl * 2 + b
                    vt = vtmp_pool.tile([P, QCH], BF16)
                    nc.vector.tensor_copy(vt[:], ps[:])
                    for blk in range(QCH // P):
                        pt = vtps.tile([P, P], BF16)
                        nc.tensor.transpose(pt[:], vt[:, blk * P:(blk + 1) * P],
                                            ident_sb[:])
                        st = (sc % N_QC) * (QCH // P) + blk
                        nc.vector.tensor_copy(v_sb[:, bh, st, :], pt[:])

        # ---- Phase 3: attention + pipelined AllGather + projection -------
        ag_in = [dram.tile([H_LOC * Dh, QCH], BF16, name=f"agi{j}")
                 for j in range(N_SC)]
        ag_out = [dram.tile([C, QCH], BF16, name=f"ago{j}")
                  for j in range(N_SC)]
        for b in range(B):
            for qc in range(N_QC):
                chunk = b * N_QC + qc
                n_kt = (QCH // P) * (qc + 1)
                for hl in range(H_LOC):
                    bh = hl * 2 + b
                    po = ops_pool.tile([P, QCH], F32, tag="po")
                    pd = dps_pool.tile([P, QCH], F32, tag="pd")
                    for kt in range(n_kt):
                        pscore = mmps.tile([P, QCH], F32, tag="mm")
                        nc.tensor.matmul(
                            pscore[:],
                            lhsT=k_sb[:, bh, kt * P:(kt + 1) * P],
                            rhs=q_sb[:, bh, qc * QCH:(qc + 1) * QCH],
                            start=True, stop=True,
                        )
                        pr = probs_pool.tile([P, QCH], BF16, tag="pr")
                        nc.scalar.activation(
                            pr[:], pscore[:],
                            mybir.ActivationFunctionType.Exp,
                            scale=SCALE,
                        )
                        j = kt - (QCH // P) * qc
                        if j >= 0:
                            nc.vector.tensor_tensor(
                                pr[:], pr[:], masks_sb[:, j, :],
                                mybir.AluOpType.mult,
                            )
                        nc.tensor.matmul(
                            po[:], lhsT=v_sb[:, bh, kt, :], rhs=pr[:],
                            start=(kt == 0), stop=(kt == n_kt - 1),
                        )
                        nc.tensor.matmul(
                            pd[:], lhsT=ones_sb[:], rhs=pr[:],
                            start=(kt == 0), stop=(kt == n_kt - 1),
                        )
                    recip = attn_pool.tile([P, QCH], F32, tag="rec")
                    nc.vector.reciprocal(recip[:], pd[:])
                    at = attn_pool.tile([P, QCH], BF16, tag="at")
                    nc.vector.tensor_tensor(
                        at[:], po[:], recip[:], mybir.AluOpType.mult
                    )
                    nc.scalar.dma_start(
                        ag_in[chunk][hl * Dh:(hl + 1) * Dh, :], at[:]
                    )

                nc.gpsimd.collective_compute(
                    "AllGather",
                    mybir.AluOpType.bypass,
                    replica_groups=[list(range(N_CORES))],
                    ins=[ag_in[chunk][:].opt()],
                    outs=[ag_out[chunk][:].opt()],
                )

                # Projection for this chunk (column-parallel Wproj)
                gt = gt_pool.tile([P, N_CC, QCH], BF16)
                nc.scalar.dma_start(
                    gt[:], ag_out[chunk][:].rearrange("(o p) q -> p o q", p=P)
                )
                for ct in range(CO_LOC // P):
                    ps = mmps.tile([P, QCH], F32, tag="mm")
                    for cc in range(N_CC):
                        nc.tensor.matmul(
                            ps[:],
                            lhsT=wproj_sb[:, cc, ct * P:(ct + 1) * P],
                            rhs=gt[:, cc, :],
                            start=(cc == 0),
                            stop=(cc == N_CC - 1),
                        )
                    ot = outp_pool.tile([P, QCH], F32)
                    nc.vector.tensor_copy(ot[:], ps[:])
                    nc.scalar.dma_start(
                        out_ext[ct * P:(ct + 1) * P,
                                chunk * QCH:(chunk + 1) * QCH],
                        ot[:],
                    )

    nc.finalize()
    return nc


_NC_CACHE = None


def _get_nc():
    global _NC_CACHE
    if _NC_CACHE is None:
        _NC_CACHE = build_nc()
    return _NC_CACHE


def make_in_maps(x, Wqkv, Wproj):
    """Shard the full inputs across the 8 cores (host side)."""
    x2 = np.ascontiguousarray(np.asarray(x, dtype=np.float32).reshape(BS, C))
    Wqkv = np.asarray(Wqkv, dtype=np.float32)
    Wproj = np.asarray(Wproj, dtype=np.float32)
    cos_t, sin_t, masks, ident, ones = _host_constants()
    in_maps = []
    for i in range(N_CORES):
        h0 = H_LOC * i
        cols = []
        for part in range(3):  # k, q, v blocks (k first per reference)
            base = part * C + h0 * Dh
            cols.append(Wqkv[:, base:base + H_LOC * Dh])
        wqkv_loc = np.ascontiguousarray(np.concatenate(cols, axis=1))
        wproj_loc = np.ascontiguousarray(Wproj[:, i * CO_LOC:(i + 1) * CO_LOC])
        in_maps.append({
            "x": x2,
            "wqkv": wqkv_loc,
            "wproj": wproj_loc,
            "cos_t": cos_t,
            "sin_t": sin_t,
            "masks": masks,
            "ident": ident,
            "ones": ones,
        })
    return in_maps


def assemble_output(results):
    outT = np.concatenate([results[i]["outT"] for i in range(N_CORES)], axis=0)
    return np.ascontiguousarray(outT.T).reshape(B, S, C).astype(np.float32)


def kernel(x, Wqkv, Wproj):
    nc = _get_nc()
    in_maps = make_in_maps(x, Wqkv, Wproj)
    res = run_bass_kernel_spmd(nc, in_maps, core_ids=list(range(N_CORES)))
    return assemble_output(res.results)


# revision 6
# speedup vs baseline: 1.2623x; 1.2623x over previous
"""Distributed FlashRotarySelfAttention kernel for 8 TRN2 NeuronCores.

Reference computation (per nn_FlashRotarySelfAttention):
  qkv = x @ Wqkv;  k, q, v = split(qkv, 3)  [k first!]
  k, q = rope(k), rope(q)
  out = causal_softmax(q k^T / sqrt(Dh)) @ v
  return out @ Wproj

Sharding: tensor-parallel over heads. Core i owns heads {2i, 2i+1}:
  - column-parallel Wqkv (k|q|v columns of its 2 heads)
  - attention fully local per (batch, head)
  - one AllGather per batch of the attention outputs (transposed,
    c-major); batch 0's gather overlaps batch 1's attention compute
  - column-parallel Wproj: each core computes 256 output channels
Host concatenates + transposes the per-core outputs.

All matmuls run in bf16 with fp32 PSUM accumulation. Softmax skips the
max-subtraction (scores are O(10) here, exp is safe in fp32); the
denominator comes from a ones-matmul that replicates it across all 128
partitions so the normalization is a plain aligned elementwise multiply.
"""

from contextlib import ExitStack

import numpy as np
import ml_dtypes

import concourse.bacc as bacc
import concourse.mybir as mybir
import concourse.tile as tile
from concourse.bass_utils import run_bass_kernel_spmd

# Problem shapes (hardcoded per contest rules).
B, S, C, H = 2, 2048, 2048, 16
Dh = C // H                      # 128
BS = B * S                       # 4096
N_CORES = 8
H_LOC = H // N_CORES             # 2 heads per core
W_LOC = 3 * H_LOC * Dh           # 768 local qkv columns
CO_LOC = C // N_CORES            # 256 output channels per core
ROPE_THETA = 10000.0
SCALE = float(Dh) ** -0.5

F32 = mybir.dt.float32
BF16 = mybir.dt.bfloat16

P = 128            # partitions
QCH = 512          # q-chunk (matmul free dim)
N_CC = C // P      # 16 contraction chunks
N_QC = S // QCH    # 4 q-chunks per batch
N_KT = S // P      # 16 k-tiles per batch

# x is cast+transposed in staged chunks: small first so the PE can start
# almost immediately, 512-row steady state after.
CHUNK_ROWS = [128, 128, 128, 128] + [512] * 7
assert sum(CHUNK_ROWS) == BS


def _host_constants():
    """Input-independent tables computed on host (compile-time constants)."""
    half = Dh // 2
    inv_freq = 1.0 / (ROPE_THETA ** (np.arange(0, half, dtype=np.float64) / half))
    ang = np.arange(S, dtype=np.float64)[None, :] * inv_freq[:, None]   # [64, S]
    cos_t = np.cos(ang).astype(np.float32)
    sin_t = np.sin(ang).astype(np.float32)
    # Causal 0/1 masks for diagonal score tiles, scoresT layout [k_local, q_local].
    # Tile j (k-tile index j within the q-chunk): keep iff q_local >= 128*j + k_local.
    kk = np.arange(P)[:, None]
    qq = np.arange(QCH)[None, :]
    masks = np.stack(
        [(qq >= P * j + kk) for j in range(4)], axis=0
    ).astype(ml_dtypes.bfloat16)                                        # [4, 128, 512]
    ident = np.eye(P, dtype=ml_dtypes.bfloat16)
    ones = np.ones((P, P), dtype=ml_dtypes.bfloat16)
    return cos_t, sin_t, masks, ident, ones


def build_nc():
    nc = bacc.Bacc(None, num_devices=N_CORES)

    x_in = nc.declare_dram_parameter("x", [BS, C], F32, isOutput=False)
    wqkv_in = nc.declare_dram_parameter("wqkv", [C, W_LOC], F32, isOutput=False)
    wproj_in = nc.declare_dram_parameter("wproj", [C, CO_LOC], F32, isOutput=False)
    cos_in = nc.declare_dram_parameter("cos_t", [Dh // 2, S], F32, isOutput=False)
    sin_in = nc.declare_dram_parameter("sin_t", [Dh // 2, S], F32, isOutput=False)
    masks_in = nc.declare_dram_parameter("masks", [4, P, QCH], BF16, isOutput=False)
    ident_in = nc.declare_dram_parameter("ident", [P, P], BF16, isOutput=False)
    ones_in = nc.declare_dram_parameter("ones", [P, P], BF16, isOutput=False)
    out_ext = nc.declare_dram_parameter("outT", [CO_LOC, BS], F32, isOutput=True)

    with tile.TileContext(nc) as tc, ExitStack() as ctx:
        consts = ctx.enter_context(tc.tile_pool(name="consts", bufs=1))
        qkvp = ctx.enter_context(tc.tile_pool(name="qkvp", bufs=1))
        xt_pool = ctx.enter_context(tc.tile_pool(name="xt", bufs=2))
        rope_pool = ctx.enter_context(tc.tile_pool(name="rope", bufs=8))
        probs_pool = ctx.enter_context(tc.tile_pool(name="probs", bufs=4))
        vtmp_pool = ctx.enter_context(tc.tile_pool(name="vtmp", bufs=2))
        attn_pool = ctx.enter_context(tc.tile_pool(name="attn", bufs=2))
        gt_pool = ctx.enter_context(tc.tile_pool(name="gt", bufs=2))
        outp_pool = ctx.enter_context(tc.tile_pool(name="outp", bufs=2))
        dram = ctx.enter_context(tc.tile_pool(name="dram", bufs=1, space="DRAM"))
        mmps = ctx.enter_context(tc.tile_pool(name="mmps", bufs=2, space="PSUM"))
        ops_pool = ctx.enter_context(tc.tile_pool(name="ops", bufs=2, space="PSUM"))
        dps_pool = ctx.enter_context(tc.tile_pool(name="dps", bufs=2, space="PSUM"))
        vtps = ctx.enter_context(tc.tile_pool(name="vtps", bufs=2, space="PSUM"))

        # ---- Startup: stage DMAs so the PE can start ASAP ----------------
        # 1) first slice of wqkv + first x micro-chunk, then the rest.
        wqkv_sb = consts.tile([P, N_CC, W_LOC], BF16)
        wqkv_src = wqkv_in.rearrange("(o p) w -> p o w", p=P)
        nc.gpsimd.dma_start(wqkv_sb[:, 0:4, :], wqkv_src[:, 0:4, :])

        x_chunks = [
            dram.tile([rows, C], BF16, name=f"xch{j}")
            for j, rows in enumerate(CHUNK_ROWS)
        ]
        row_off = [0]
        for rows in CHUNK_ROWS:
            row_off.append(row_off[-1] + rows)
        nc.gpsimd.dma_start(x_chunks[0][:], x_in[0:CHUNK_ROWS[0], :])

        nc.gpsimd.dma_start(wqkv_sb[:, 4:16, :], wqkv_src[:, 4:16, :])

        cos_sb = consts.tile([Dh // 2, S], F32)
        nc.scalar.dma_start(cos_sb[:], cos_in[:])
        sin_sb = consts.tile([Dh // 2, S], F32)
        nc.scalar.dma_start(sin_sb[:], sin_in[:])
        masks_sb = consts.tile([P, 4, QCH], BF16)
        nc.scalar.dma_start(masks_sb[:], masks_in.rearrange("j p q -> p j q"))
        ident_sb = consts.tile([P, P], BF16)
        nc.scalar.dma_start(ident_sb[:], ident_in[:])
        ones_sb = consts.tile([P, P], BF16)
        nc.scalar.dma_start(ones_sb[:], ones_in[:])

        for j in range(1, len(CHUNK_ROWS)):
            nc.gpsimd.dma_start(x_chunks[j][:], x_in[row_off[j]:row_off[j + 1], :])

        # wproj only needed at projection time, keep it off the startup path
        wproj_sb = consts.tile([P, N_CC, CO_LOC], BF16)
        nc.gpsimd.dma_start(wproj_sb[:], wproj_in.rearrange("(o p) w -> p o w", p=P))

        # Resident activations: d-major q/k, k-major v. bh = h_local*2 + b
        q_sb = qkvp.tile([P, 2 * H_LOC, S], BF16)
        k_sb = qkvp.tile([P, 2 * H_LOC, S], BF16)
        v_sb = qkvp.tile([P, 2 * H_LOC, N_KT, Dh], BF16)

        # ---- Phase 2: transpose-load x^T, QKV matmuls, RoPE --------------
        for j, rows in enumerate(CHUNK_ROWS):
            g0 = row_off[j]              # global row offset in [0, BS)
            b = g0 // S
            s0 = g0 - b * S              # position offset within batch
            cos_c = cos_sb[:, s0:s0 + rows]
            sin_c = sin_sb[:, s0:s0 + rows]
            # x^T tile [c_in(128, o), s(rows)] via XBAR transpose
            xt = xt_pool.tile([P, N_CC, QCH], BF16, tag="xt")
            xtv = xt[:, :, :rows]
            nc.sync.dma_start_transpose(xtv, x_chunks[j][:])

            for ct in range(6):
                ps = mmps.tile([P, QCH], F32, tag="mm")
                psv = ps[:, :rows]
                for cc in range(N_CC):
                    nc.tensor.matmul(
                        psv,
                        lhsT=wqkv_sb[:, cc, ct * P:(ct + 1) * P],
                        rhs=xtv[:, cc, :],
                        start=(cc == 0),
                        stop=(cc == N_CC - 1),
                    )
                if ct < 4:
                    # k (ct 0,1) and q (ct 2,3): RoPE -> bf16 resident
                    hl = ct % 2
                    dst = k_sb if ct < 2 else q_sb
                    bh = hl * 2 + b
                    lo = psv[0:64, :]
                    hi = psv[64:128, :]
                    t1 = rope_pool.tile([64, QCH], F32, tag="rt")
                    t2 = rope_pool.tile([64, QCH], F32, tag="rt")
                    t3 = rope_pool.tile([64, QCH], F32, tag="rt")
                    t4 = rope_pool.tile([64, QCH], F32, tag="rt")
                    nc.any.tensor_tensor(t1[:, :rows], lo, cos_c,
                                         mybir.AluOpType.mult)
                    nc.any.tensor_tensor(t2[:, :rows], hi, sin_c,
                                         mybir.AluOpType.mult)
                    nc.any.tensor_tensor(
                        dst[0:64, bh, s0:s0 + rows],
                        t1[:, :rows], t2[:, :rows], mybir.AluOpType.subtract,
                    )
                    nc.any.tensor_tensor(t3[:, :rows], hi, cos_c,
                                         mybir.AluOpType.mult)
                    nc.any.tensor_tensor(t4[:, :rows], lo, sin_c,
                                         mybir.AluOpType.mult)
                    nc.any.tensor_tensor(
                        dst[64:128, bh, s0:s0 + rows],
                        t3[:, :rows], t4[:, :rows], mybir.AluOpType.add,
                    )
                else:
                    # v (ct 4,5): cast to bf16, PE-transpose to k-major
                    hl = ct - 4
                    bh = hl * 2 + b
                    vt = vtmp_pool.tile([P, QCH], BF16, tag="vt")
                    nc.vector.tensor_copy(vt[:, :rows], psv)
                    for blk in range(rows // P):
                        pt = vtps.tile([P, P], BF16)
                        nc.tensor.transpose(pt[:], vt[:, blk * P:(blk + 1) * P],
                                            ident_sb[:])
                        st = s0 // P + blk
                        nc.vector.tensor_copy(v_sb[:, bh, st, :], pt[:])

        # ---- Phase 3: attention; per-batch AllGather + projection --------
        ag_in = [dram.tile([H_LOC * Dh, S], BF16, name=f"agi{j}")
                 for j in range(B)]
        ag_out = [dram.tile([C, S], BF16, name=f"ago{j}") for j in range(B)]

        def attention(b):
            for qc in range(N_QC):
                n_kt = (QCH // P) * (qc + 1)
                for hl in range(H_LOC):
                    bh = hl * 2 + b
                    po = ops_pool.tile([P, QCH], F32, tag="po")
                    pd = dps_pool.tile([P, QCH], F32, tag="pd")
                    for kt in range(n_kt):
                        pscore = mmps.tile([P, QCH], F32, tag="mm")
                        nc.tensor.matmul(
                            pscore[:],
                            lhsT=k_sb[:, bh, kt * P:(kt + 1) * P],
                            rhs=q_sb[:, bh, qc * QCH:(qc + 1) * QCH],
                            start=True, stop=True,
                        )
                        pr = probs_pool.tile([P, QCH], BF16, tag="pr")
                        nc.scalar.activation(
                            pr[:], pscore[:],
                            mybir.ActivationFunctionType.Exp,
                            scale=SCALE,
                        )
                        jj = kt - (QCH // P) * qc
                        if jj >= 0:
                            nc.vector.tensor_tensor(
                                pr[:], pr[:], masks_sb[:, jj, :],
                                mybir.AluOpType.mult,
                            )
                        nc.tensor.matmul(
                            po[:], lhsT=v_sb[:, bh, kt, :], rhs=pr[:],
                            start=(kt == 0), stop=(kt == n_kt - 1),
                        )
                        nc.tensor.matmul(
                            pd[:], lhsT=ones_sb[:], rhs=pr[:],
                            start=(kt == 0), stop=(kt == n_kt - 1),
                        )
                    recip = attn_pool.tile([P, QCH], F32, tag="rec")
                    nc.vector.reciprocal(recip[:], pd[:])
                    at = attn_pool.tile([P, QCH], BF16, tag="at")
                    nc.vector.tensor_tensor(
                        at[:], po[:], recip[:], mybir.AluOpType.mult
                    )
                    nc.scalar.dma_start(
                        ag_in[b][hl * Dh:(hl + 1) * Dh,
                                 qc * QCH:(qc + 1) * QCH],
                        at[:],
                    )

        def allgather(b):
            nc.gpsimd.collective_compute(
                "AllGather",
                mybir.AluOpType.bypass,
                replica_groups=[list(range(N_CORES))],
                ins=[ag_in[b][:].opt()],
                outs=[ag_out[b][:].opt()],
            )

        def projection(b):
            for qc in range(N_QC):
                gt = gt_pool.tile([P, N_CC, QCH], BF16, tag="gt")
                nc.scalar.dma_start(
                    gt[:],
                    ag_out[b][:, qc * QCH:(qc + 1) * QCH].rearrange(
                        "(o p) q -> p o q", p=P
                    ),
                )
                for ct in range(CO_LOC // P):
                    ps = mmps.tile([P, QCH], F32, tag="mm")
                    for cc in range(N_CC):
                        nc.tensor.matmul(
                            ps[:],
                            lhsT=wproj_sb[:, cc, ct * P:(ct + 1) * P],
                            rhs=gt[:, cc, :],
                            start=(cc == 0),
                            stop=(cc == N_CC - 1),
                        )
                    ot = outp_pool.tile([P, QCH], F32, tag="ot")
                    nc.vector.tensor_copy(ot[:], ps[:])
                    nc.scalar.dma_start(
                        out_ext[ct * P:(ct + 1) * P,
                                b * S + qc * QCH:b * S + (qc + 1) * QCH],
                        ot[:],
                    )

        attention(0)
        allgather(0)
        attention(1)
        allgather(1)
        projection(0)
        projection(1)

    nc.finalize()
    return nc


_NC_CACHE = None


def _get_nc():
    global _NC_CACHE
    if _NC_CACHE is None:
        _NC_CACHE = build_nc()
    return _NC_CACHE


def make_in_maps(x, Wqkv, Wproj):
    """Shard the full inputs across the 8 cores (host side)."""
    x2 = np.ascontiguousarray(np.asarray(x, dtype=np.float32).reshape(BS, C))
    Wqkv = np.asarray(Wqkv, dtype=np.float32)
    Wproj = np.asarray(Wproj, dtype=np.float32)
    cos_t, sin_t, masks, ident, ones = _host_constants()
    in_maps = []
    for i in range(N_CORES):
        h0 = H_LOC * i
        cols = []
        for part in range(3):  # k, q, v blocks (k first per reference)
            base = part * C + h0 * Dh
            cols.append(Wqkv[:, base:base + H_LOC * Dh])
        wqkv_loc = np.ascontiguousarray(np.concatenate(cols, axis=1))
        wproj_loc = np.ascontiguousarray(Wproj[:, i * CO_LOC:(i + 1) * CO_LOC])
        in_maps.append({
            "x": x2,
            "wqkv": wqkv_loc,
            "wproj": wproj_loc,
            "cos_t": cos_t,
            "sin_t": sin_t,
            "masks": masks,
            "ident": ident,
            "ones": ones,
        })
    return in_maps


def assemble_output(results):
    outT = np.concatenate([results[i]["outT"] for i in range(N_CORES)], axis=0)
    return np.ascontiguousarray(outT.T).reshape(B, S, C).astype(np.float32)


def kernel(x, Wqkv, Wproj):
    nc = _get_nc()
    in_maps = make_in_maps(x, Wqkv, Wproj)
    res = run_bass_kernel_spmd(nc, in_maps, core_ids=list(range(N_CORES)))
    return assemble_output(res.results)


# revision 7
# speedup vs baseline: 1.2854x; 1.0183x over previous
"""Distributed FlashRotarySelfAttention kernel for 8 TRN2 NeuronCores.

Reference computation (per nn_FlashRotarySelfAttention):
  qkv = x @ Wqkv;  k, q, v = split(qkv, 3)  [k first!]
  k, q = rope(k), rope(q)
  out = causal_softmax(q k^T / sqrt(Dh)) @ v
  return out @ Wproj

Sharding: tensor-parallel over heads. Core i owns heads {2i, 2i+1}:
  - column-parallel Wqkv (k|q|v columns of its 2 heads)
  - attention fully local per (batch, head)
  - one AllGather per batch of the attention outputs (transposed,
    c-major); batch 0's gather overlaps batch 1's attention compute
  - column-parallel Wproj: each core computes 256 output channels
Host concatenates + transposes the per-core outputs.

All matmuls run in bf16 with fp32 PSUM accumulation. Softmax skips the
max-subtraction (scores are O(10) here, exp is safe in fp32); the
denominator comes from a ones-matmul that replicates it across all 128
partitions so the normalization is a plain aligned elementwise multiply.
"""

from contextlib import ExitStack

import numpy as np
import ml_dtypes

import concourse.bacc as bacc
import concourse.mybir as mybir
import concourse.tile as tile
from concourse.bass_utils import run_bass_kernel_spmd

# Problem shapes (hardcoded per contest rules).
B, S, C, H = 2, 2048, 2048, 16
Dh = C // H                      # 128
BS = B * S                       # 4096
N_CORES = 8
H_LOC = H // N_CORES             # 2 heads per core
W_LOC = 3 * H_LOC * Dh           # 768 local qkv columns
CO_LOC = C // N_CORES            # 256 output channels per core
ROPE_THETA = 10000.0
SCALE = float(Dh) ** -0.5

F32 = mybir.dt.float32
BF16 = mybir.dt.bfloat16

P = 128            # partitions
QCH = 512          # q-chunk (matmul free dim)
N_CC = C // P      # 16 contraction chunks
N_QC = S // QCH    # 4 q-chunks per batch
N_KT = S // P      # 16 k-tiles per batch

# x is cast+transposed in staged chunks: small first so the PE can start
# almost immediately, 512-row steady state after.
CHUNK_ROWS = [128, 128, 128, 128] + [512] * 7
assert sum(CHUNK_ROWS) == BS


def _host_constants():
    """Input-independent tables computed on host (compile-time constants)."""
    half = Dh // 2
    inv_freq = 1.0 / (ROPE_THETA ** (np.arange(0, half, dtype=np.float64) / half))
    ang = np.arange(S, dtype=np.float64)[None, :] * inv_freq[:, None]   # [64, S]
    cos_t = np.cos(ang).astype(np.float32)
    sin_t = np.sin(ang).astype(np.float32)
    # Causal 0/1 masks for diagonal score tiles, scoresT layout [k_local, q_local].
    # Tile j (k-tile index j within the q-chunk): keep iff q_local >= 128*j + k_local.
    kk = np.arange(P)[:, None]
    qq = np.arange(QCH)[None, :]
    masks = np.stack(
        [(qq >= P * j + kk) for j in range(4)], axis=0
    ).astype(ml_dtypes.bfloat16)                                        # [4, 128, 512]
    ident = np.eye(P, dtype=ml_dtypes.bfloat16)
    ones = np.ones((P, P), dtype=np.float32)
    return cos_t, sin_t, masks, ident, ones


def build_nc():
    nc = bacc.Bacc(None, num_devices=N_CORES)

    x_in = nc.declare_dram_parameter("x", [BS, C], F32, isOutput=False)
    wqkv_in = nc.declare_dram_parameter("wqkv", [C, W_LOC], F32, isOutput=False)
    wproj_in = nc.declare_dram_parameter("wproj", [C, CO_LOC], F32, isOutput=False)
    cos_in = nc.declare_dram_parameter("cos_t", [Dh // 2, S], F32, isOutput=False)
    sin_in = nc.declare_dram_parameter("sin_t", [Dh // 2, S], F32, isOutput=False)
    masks_in = nc.declare_dram_parameter("masks", [4, P, QCH], BF16, isOutput=False)
    ident_in = nc.declare_dram_parameter("ident", [P, P], BF16, isOutput=False)
    ones_in = nc.declare_dram_parameter("ones", [P, P], F32, isOutput=False)
    out_ext = nc.declare_dram_parameter("outT", [CO_LOC, BS], F32, isOutput=True)

    with tile.TileContext(nc) as tc, ExitStack() as ctx:
        consts = ctx.enter_context(tc.tile_pool(name="consts", bufs=1))
        qkvp = ctx.enter_context(tc.tile_pool(name="qkvp", bufs=1))
        xt_pool = ctx.enter_context(tc.tile_pool(name="xt", bufs=2))
        rope_pool = ctx.enter_context(tc.tile_pool(name="rope", bufs=8))
        probs_pool = ctx.enter_context(tc.tile_pool(name="probs", bufs=6))
        vtmp_pool = ctx.enter_context(tc.tile_pool(name="vtmp", bufs=2))
        attn_pool = ctx.enter_context(tc.tile_pool(name="attn", bufs=2))
        gt_pool = ctx.enter_context(tc.tile_pool(name="gt", bufs=2))
        outp_pool = ctx.enter_context(tc.tile_pool(name="outp", bufs=2))
        dram = ctx.enter_context(tc.tile_pool(name="dram", bufs=1, space="DRAM"))
        mmps = ctx.enter_context(tc.tile_pool(name="mmps", bufs=2, space="PSUM"))
        sps_pool = ctx.enter_context(tc.tile_pool(name="sps", bufs=3, space="PSUM"))
        ops_pool = ctx.enter_context(tc.tile_pool(name="ops", bufs=1, space="PSUM"))
        vtps = ctx.enter_context(tc.tile_pool(name="vtps", bufs=2, space="PSUM"))

        # ---- Startup: stage DMAs so the PE can start ASAP ----------------
        # 1) first slice of wqkv + first x micro-chunk, then the rest.
        wqkv_sb = consts.tile([P, N_CC, W_LOC], BF16)
        wqkv_src = wqkv_in.rearrange("(o p) w -> p o w", p=P)
        nc.gpsimd.dma_start(wqkv_sb[:, 0:4, :], wqkv_src[:, 0:4, :])

        x_chunks = [
            dram.tile([rows, C], BF16, name=f"xch{j}")
            for j, rows in enumerate(CHUNK_ROWS)
        ]
        row_off = [0]
        for rows in CHUNK_ROWS:
            row_off.append(row_off[-1] + rows)
        nc.gpsimd.dma_start(x_chunks[0][:], x_in[0:CHUNK_ROWS[0], :])

        nc.gpsimd.dma_start(wqkv_sb[:, 4:16, :], wqkv_src[:, 4:16, :])

        cos_sb = consts.tile([Dh // 2, S], F32)
        nc.scalar.dma_start(cos_sb[:], cos_in[:])
        sin_sb = consts.tile([Dh // 2, S], F32)
        nc.scalar.dma_start(sin_sb[:], sin_in[:])
        masks_sb = consts.tile([P, 4, QCH], BF16)
        nc.scalar.dma_start(masks_sb[:], masks_in.rearrange("j p q -> p j q"))
        ident_sb = consts.tile([P, P], BF16)
        nc.scalar.dma_start(ident_sb[:], ident_in[:])
        ones_sb = consts.tile([P, P], F32)
        nc.scalar.dma_start(ones_sb[:], ones_in[:])

        for j in range(1, len(CHUNK_ROWS)):
            nc.gpsimd.dma_start(x_chunks[j][:], x_in[row_off[j]:row_off[j + 1], :])

        # wproj only needed at projection time, keep it off the startup path
        wproj_sb = consts.tile([P, N_CC, CO_LOC], BF16)
        nc.gpsimd.dma_start(wproj_sb[:], wproj_in.rearrange("(o p) w -> p o w", p=P))

        # Resident activations: d-major q/k, k-major v. bh = h_local*2 + b
        q_sb = qkvp.tile([P, 2 * H_LOC, S], BF16)
        k_sb = qkvp.tile([P, 2 * H_LOC, S], BF16)
        v_sb = qkvp.tile([P, 2 * H_LOC, N_KT, Dh], BF16)

        # ---- Phase 2: transpose-load x^T, QKV matmuls, RoPE --------------
        for j, rows in enumerate(CHUNK_ROWS):
            g0 = row_off[j]              # global row offset in [0, BS)
            b = g0 // S
            s0 = g0 - b * S              # position offset within batch
            cos_c = cos_sb[:, s0:s0 + rows]
            sin_c = sin_sb[:, s0:s0 + rows]
            # x^T tile [c_in(128, o), s(rows)] via XBAR transpose
            xt = xt_pool.tile([P, N_CC, QCH], BF16, tag="xt")
            xtv = xt[:, :, :rows]
            nc.sync.dma_start_transpose(xtv, x_chunks[j][:])

            for ct in range(6):
                ps = mmps.tile([P, QCH], F32, tag="mm")
                psv = ps[:, :rows]
                for cc in range(N_CC):
                    nc.tensor.matmul(
                        psv,
                        lhsT=wqkv_sb[:, cc, ct * P:(ct + 1) * P],
                        rhs=xtv[:, cc, :],
                        start=(cc == 0),
                        stop=(cc == N_CC - 1),
                    )
                if ct < 4:
                    # k (ct 0,1) and q (ct 2,3): RoPE -> bf16 resident
                    hl = ct % 2
                    dst = k_sb if ct < 2 else q_sb
                    bh = hl * 2 + b
                    lo = psv[0:64, :]
                    hi = psv[64:128, :]
                    t1 = rope_pool.tile([64, QCH], F32, tag="rt")
                    t2 = rope_pool.tile([64, QCH], F32, tag="rt")
                    t3 = rope_pool.tile([64, QCH], F32, tag="rt")
                    t4 = rope_pool.tile([64, QCH], F32, tag="rt")
                    nc.any.tensor_tensor(t1[:, :rows], lo, cos_c,
                                         mybir.AluOpType.mult)
                    nc.any.tensor_tensor(t2[:, :rows], hi, sin_c,
                                         mybir.AluOpType.mult)
                    nc.any.tensor_tensor(
                        dst[0:64, bh, s0:s0 + rows],
                        t1[:, :rows], t2[:, :rows], mybir.AluOpType.subtract,
                    )
                    nc.any.tensor_tensor(t3[:, :rows], hi, cos_c,
                                         mybir.AluOpType.mult)
                    nc.any.tensor_tensor(t4[:, :rows], lo, sin_c,
                                         mybir.AluOpType.mult)
                    nc.any.tensor_tensor(
                        dst[64:128, bh, s0:s0 + rows],
                        t3[:, :rows], t4[:, :rows], mybir.AluOpType.add,
                    )
                else:
                    # v (ct 4,5): cast to bf16, PE-transpose to k-major
                    hl = ct - 4
                    bh = hl * 2 + b
                    vt = vtmp_pool.tile([P, QCH], BF16, tag="vt")
                    nc.vector.tensor_copy(vt[:, :rows], psv)
                    for blk in range(rows // P):
                        pt = vtps.tile([P, P], BF16)
                        nc.tensor.transpose(pt[:], vt[:, blk * P:(blk + 1) * P],
                                            ident_sb[:])
                        st = s0 // P + blk
                        nc.vector.tensor_copy(v_sb[:, bh, st, :], pt[:])

        # ---- Phase 3: attention; per-batch AllGather + projection --------
        ag_in = [dram.tile([H_LOC * Dh, S], BF16, name=f"agi{j}")
                 for j in range(B)]
        ag_out = [dram.tile([C, S], BF16, name=f"ago{j}") for j in range(B)]

        def attention(b):
            for qc in range(N_QC):
                n_kt = (QCH // P) * (qc + 1)
                for hl in range(H_LOC):
                    bh = hl * 2 + b
                    po = ops_pool.tile([P, QCH], F32, tag="po")
                    acc = attn_pool.tile([P, QCH], F32, tag="acc")
                    for kt in range(n_kt):
                        jj = kt - (QCH // P) * qc
                        # diagonal tiles: columns below 128*jj are fully
                        # masked -- skip computing them entirely
                        off = P * jj if jj > 0 else 0
                        w = QCH - off
                        pscore = sps_pool.tile([P, QCH], F32, tag="sc")
                        nc.tensor.matmul(
                            pscore[:, off:],
                            lhsT=k_sb[:, bh, kt * P:(kt + 1) * P],
                            rhs=q_sb[:, bh, qc * QCH + off:(qc + 1) * QCH],
                            start=True, stop=True,
                        )
                        pr = probs_pool.tile([P, QCH], BF16, tag="pr")
                        nc.scalar.activation(
                            pr[:, off:], pscore[:, off:],
                            mybir.ActivationFunctionType.Exp,
                            scale=SCALE,
                        )
                        if jj >= 0:
                            nc.vector.tensor_tensor(
                                pr[:, off:], pr[:, off:],
                                masks_sb[:, jj, off:],
                                mybir.AluOpType.mult,
                            )
                        if kt == 0:
                            nc.vector.tensor_copy(acc[:], pr[:])
                        else:
                            nc.vector.tensor_tensor(
                                acc[:, off:], acc[:, off:], pr[:, off:],
                                mybir.AluOpType.add,
                            )
                        nc.tensor.matmul(
                            po[:, off:], lhsT=v_sb[:, bh, kt, :],
                            rhs=pr[:, off:],
                            start=(kt == 0), stop=(kt == n_kt - 1),
                        )
                    pd = mmps.tile([P, QCH], F32, tag="mm")
                    nc.tensor.matmul(
                        pd[:], lhsT=ones_sb[:], rhs=acc[:],
                        start=True, stop=True,
                    )
                    recip = attn_pool.tile([P, QCH], F32, tag="rec")
                    nc.vector.reciprocal(recip[:], pd[:])
                    at = attn_pool.tile([P, QCH], BF16, tag="at")
                    nc.vector.tensor_tensor(
                        at[:], po[:], recip[:], mybir.AluOpType.mult
                    )
                    nc.scalar.dma_start(
                        ag_in[b][hl * Dh:(hl + 1) * Dh,
                                 qc * QCH:(qc + 1) * QCH],
                        at[:],
                    )

        def allgather(b):
            nc.gpsimd.collective_compute(
                "AllGather",
                mybir.AluOpType.bypass,
                replica_groups=[list(range(N_CORES))],
                ins=[ag_in[b][:].opt()],
                outs=[ag_out[b][:].opt()],
            )

        def projection(b):
            for qc in range(N_QC):
                gt = gt_pool.tile([P, N_CC, QCH], BF16, tag="gt")
                nc.scalar.dma_start(
                    gt[:],
                    ag_out[b][:, qc * QCH:(qc + 1) * QCH].rearrange(
                        "(o p) q -> p o q", p=P
                    ),
                )
                for ct in range(CO_LOC // P):
                    ps = mmps.tile([P, QCH], F32, tag="mm")
                    for cc in range(N_CC):
                        nc.tensor.matmul(
                            ps[:],
                            lhsT=wproj_sb[:, cc, ct * P:(ct + 1) * P],
                            rhs=gt[:, cc, :],
                            start=(cc == 0),
                            stop=(cc == N_CC - 1),
                        )
                    ot = outp_pool.tile([P, QCH], F32, tag="ot")
                    nc.vector.tensor_copy(ot[:], ps[:])
                    nc.scalar.dma_start(
                        out_ext[ct * P:(ct + 1) * P,
                                b * S + qc * QCH:b * S + (qc + 1) * QCH],
                        ot[:],
                    )

        attention(0)
        allgather(0)
        attention(1)
        allgather(1)
        projection(0)
        projection(1)

    nc.finalize()
    return nc


_NC_CACHE = None


def _get_nc():
    global _NC_CACHE
    if _NC_CACHE is None:
        _NC_CACHE = build_nc()
    return _NC_CACHE


def make_in_maps(x, Wqkv, Wproj):
    """Shard the full inputs across the 8 cores (host side)."""
    x2 = np.ascontiguousarray(np.asarray(x, dtype=np.float32).reshape(BS, C))
    Wqkv = np.asarray(Wqkv, dtype=np.float32)
    Wproj = np.asarray(Wproj, dtype=np.float32)
    cos_t, sin_t, masks, ident, ones = _host_constants()
    in_maps = []
    for i in range(N_CORES):
        h0 = H_LOC * i
        cols = []
        for part in range(3):  # k, q, v blocks (k first per reference)
            base = part * C + h0 * Dh
            cols.append(Wqkv[:, base:base + H_LOC * Dh])
        wqkv_loc = np.ascontiguousarray(np.concatenate(cols, axis=1))
        wproj_loc = np.ascontiguousarray(Wproj[:, i * CO_LOC:(i + 1) * CO_LOC])
        in_maps.append({
            "x": x2,
            "wqkv": wqkv_loc,
            "wproj": wproj_loc,
            "cos_t": cos_t,
            "sin_t": sin_t,
            "masks": masks,
            "ident": ident,
            "ones": ones,
        })
    return in_maps


def assemble_output(results):
    outT = np.concatenate([results[i]["outT"] for i in range(N_CORES)], axis=0)
    return np.ascontiguousarray(outT.T).reshape(B, S, C).astype(np.float32)


def kernel(x, Wqkv, Wproj):
    nc = _get_nc()
    in_maps = make_in_maps(x, Wqkv, Wproj)
    res = run_bass_kernel_spmd(nc, in_maps, core_ids=list(range(N_CORES)))
    return assemble_output(res.results)


# revision 10
# speedup vs baseline: 1.3075x; 1.0172x over previous
"""Distributed FlashRotarySelfAttention kernel for 8 TRN2 NeuronCores.

Reference computation (per nn_FlashRotarySelfAttention):
  qkv = x @ Wqkv;  k, q, v = split(qkv, 3)  [k first!]
  k, q = rope(k), rope(q)
  out = causal_softmax(q k^T / sqrt(Dh)) @ v
  return out @ Wproj

Sharding: tensor-parallel over heads. Core i owns heads {2i, 2i+1}:
  - column-parallel Wqkv (k|q|v columns of its 2 heads)
  - attention fully local per (batch, head)
  - one AllGather per batch of the attention outputs (transposed,
    c-major); batch 0's gather overlaps batch 1's attention compute
  - column-parallel Wproj: each core computes 256 output channels
Host concatenates + transposes the per-core outputs.

All matmuls run in bf16 with fp32 PSUM accumulation. x is transposed
on-chip: f32 tile load -> DVE cast to bf16 -> XBAR SBUF->SBUF transpose
(no HBM roundtrip). Softmax skips the max-subtraction (scores are O(10)
here, exp is safe in fp32); the denominator is accumulated on DVE and
reduced across partitions by a single ones-matmul per group, which also
replicates it across partitions so normalization is an aligned multiply.
"""

from contextlib import ExitStack

import numpy as np
import ml_dtypes

import concourse.bacc as bacc
import concourse.mybir as mybir
import concourse.tile as tile
from concourse.bass_utils import run_bass_kernel_spmd

# Problem shapes (hardcoded per contest rules).
B, S, C, H = 2, 2048, 2048, 16
Dh = C // H                      # 128
BS = B * S                       # 4096
N_CORES = 8
H_LOC = H // N_CORES             # 2 heads per core
W_LOC = 3 * H_LOC * Dh           # 768 local qkv columns
CO_LOC = C // N_CORES            # 256 output channels per core
ROPE_THETA = 10000.0
SCALE = float(Dh) ** -0.5

F32 = mybir.dt.float32
BF16 = mybir.dt.bfloat16

P = 128            # partitions
QCH = 512          # q-chunk (matmul free dim)
N_SC = BS // QCH   # 8 s-chunks over B*S
N_CC = C // P      # 16 contraction chunks
N_QC = S // QCH    # 4 q-chunks per batch
N_KT = S // P      # 16 k-tiles per batch


def _host_constants():
    """Input-independent tables computed on host (compile-time constants)."""
    half = Dh // 2
    inv_freq = 1.0 / (ROPE_THETA ** (np.arange(0, half, dtype=np.float64) / half))
    ang = np.arange(S, dtype=np.float64)[None, :] * inv_freq[:, None]   # [64, S]
    cos_t = np.cos(ang).astype(ml_dtypes.bfloat16)
    sin_t = np.sin(ang).astype(ml_dtypes.bfloat16)
    # Causal 0/1 masks for diagonal score tiles, scoresT layout [k_local, q_local].
    # Tile j (k-tile index j within the q-chunk): keep iff q_local >= 128*j + k_local.
    kk = np.arange(P)[:, None]
    qq = np.arange(QCH)[None, :]
    masks = np.stack(
        [(qq >= P * j + kk) for j in range(4)], axis=0
    ).astype(ml_dtypes.bfloat16)                                        # [4, 128, 512]
    ident = np.eye(P, dtype=ml_dtypes.bfloat16)
    ones = np.ones((P, P), dtype=ml_dtypes.bfloat16)
    return cos_t, sin_t, masks, ident, ones


def build_nc():
    nc = bacc.Bacc(None, num_devices=N_CORES)

    x_in = nc.declare_dram_parameter("x", [BS, C], F32, isOutput=False)
    wqkv_in = nc.declare_dram_parameter("wqkv", [C, W_LOC], F32, isOutput=False)
    wproj_in = nc.declare_dram_parameter("wproj", [C, CO_LOC], F32, isOutput=False)
    cos_in = nc.declare_dram_parameter("cos_t", [Dh // 2, S], BF16, isOutput=False)
    sin_in = nc.declare_dram_parameter("sin_t", [Dh // 2, S], BF16, isOutput=False)
    masks_in = nc.declare_dram_parameter("masks", [4, P, QCH], BF16, isOutput=False)
    ident_in = nc.declare_dram_parameter("ident", [P, P], BF16, isOutput=False)
    ones_in = nc.declare_dram_parameter("ones", [P, P], BF16, isOutput=False)
    out_ext = nc.declare_dram_parameter("outT", [CO_LOC, BS], F32, isOutput=True)

    with tile.TileContext(nc) as tc, ExitStack() as ctx:
        consts = ctx.enter_context(tc.tile_pool(name="consts", bufs=1))
        qkvp = ctx.enter_context(tc.tile_pool(name="qkvp", bufs=1))
        xf_pool = ctx.enter_context(tc.tile_pool(name="xf", bufs=2))
        xb_pool = ctx.enter_context(tc.tile_pool(name="xb", bufs=1))
        xt_pool = ctx.enter_context(tc.tile_pool(name="xt", bufs=2))
        rope_pool = ctx.enter_context(tc.tile_pool(name="rope", bufs=6))
        probs_pool = ctx.enter_context(tc.tile_pool(name="probs", bufs=5))
        vtmp_pool = ctx.enter_context(tc.tile_pool(name="vtmp", bufs=2))
        attn_pool = ctx.enter_context(tc.tile_pool(name="attn", bufs=2))
        gt_pool = ctx.enter_context(tc.tile_pool(name="gt", bufs=2))
        outp_pool = ctx.enter_context(tc.tile_pool(name="outp", bufs=2))
        dram = ctx.enter_context(tc.tile_pool(name="dram", bufs=1, space="DRAM"))
        mmps = ctx.enter_context(tc.tile_pool(name="mmps", bufs=2, space="PSUM"))
        sps_pool = ctx.enter_context(tc.tile_pool(name="sps", bufs=3, space="PSUM"))
        ops_pool = ctx.enter_context(tc.tile_pool(name="ops", bufs=1, space="PSUM"))
        vtps = ctx.enter_context(tc.tile_pool(name="vtps", bufs=2, space="PSUM"))

        # ---- Startup: first wqkv slice so the PE can start ASAP ----------
        wqkv_sb = consts.tile([P, N_CC, W_LOC], BF16)
        wqkv_src = wqkv_in.rearrange("(o p) w -> p o w", p=P)
        nc.gpsimd.dma_start(wqkv_sb[:, 0:4, :], wqkv_src[:, 0:4, :])
        nc.gpsimd.dma_start(wqkv_sb[:, 4:16, :], wqkv_src[:, 4:16, :])

        cos_sb = consts.tile([Dh // 2, S], BF16)
        nc.scalar.dma_start(cos_sb[:], cos_in[:])
        sin_sb = consts.tile([Dh // 2, S], BF16)
        nc.scalar.dma_start(sin_sb[:], sin_in[:])
        masks_sb = consts.tile([P, 4, QCH], BF16)
        nc.scalar.dma_start(masks_sb[:], masks_in.rearrange("j p q -> p j q"))
        ident_sb = consts.tile([P, P], BF16)
        nc.scalar.dma_start(ident_sb[:], ident_in[:])
        ones_sb = consts.tile([P, P], BF16)
        nc.scalar.dma_start(ones_sb[:], ones_in[:])

        wproj_sb = consts.tile([P, N_CC, CO_LOC], BF16)
        nc.gpsimd.dma_start(wproj_sb[:], wproj_in.rearrange("(o p) w -> p o w", p=P))

        # Resident activations: d-major q/k, k-major v. bh = h_local*2 + b
        q_sb = qkvp.tile([P, 2 * H_LOC, S], BF16)
        k_sb = qkvp.tile([P, 2 * H_LOC, S], BF16)
        v_sb = qkvp.tile([P, 2 * H_LOC, N_KT, Dh], BF16)

        # ---- Phase 2: x load/cast/transpose on-chip, QKV matmuls, RoPE ---
        for sc in range(N_SC):
            g0 = sc * QCH
            b = g0 // S
            s0 = g0 - b * S              # position offset within batch
            cos_c = cos_sb[:, s0:s0 + QCH]
            sin_c = sin_sb[:, s0:s0 + QCH]
            # build x^T tile [c_in(128, o), s(512)]: per 128-row slice,
            # f32 load -> bf16 cast -> XBAR SBUF->SBUF transpose
            xt = xt_pool.tile([P, N_CC, QCH], BF16, tag="xt")
            for blk in range(QCH // P):
                r0 = g0 + blk * P
                xf = xf_pool.tile([P, C], F32, tag="xf")
                nc.scalar.dma_start(xf[:], x_in[r0:r0 + P, :])
                xb = xb_pool.tile([P, C], BF16, tag="xb")
                nc.vector.tensor_copy(xb[:], xf[:])
                nc.sync.dma_start_transpose(
                    xt[:, :, blk * P:(blk + 1) * P], xb[:]
                )

            for ct in range(6):
                ps = mmps.tile([P, QCH], F32, tag="mm")
                for cc in range(N_CC):
                    nc.tensor.matmul(
                        ps[:],
                        lhsT=wqkv_sb[:, cc, ct * P:(ct + 1) * P],
                        rhs=xt[:, cc, :],
                        start=(cc == 0),
                        stop=(cc == N_CC - 1),
                    )
                if ct < 4:
                    # k (ct 0,1) and q (ct 2,3): RoPE -> bf16 resident
                    hl = ct % 2
                    dst = k_sb if ct < 2 else q_sb
                    bh = hl * 2 + b
                    lo = ps[0:64, :]
                    hi = ps[64:128, :]
                    t1 = rope_pool.tile([64, QCH], F32, tag="rt")
                    t2 = rope_pool.tile([64, QCH], F32, tag="rt")
                    t3 = rope_pool.tile([64, QCH], F32, tag="rt")
                    t4 = rope_pool.tile([64, QCH], F32, tag="rt")
                    nc.any.tensor_tensor(t1[:], lo, cos_c, mybir.AluOpType.mult)
                    nc.any.tensor_tensor(t2[:], hi, sin_c, mybir.AluOpType.mult)
                    nc.any.tensor_tensor(
                        dst[0:64, bh, s0:s0 + QCH],
                        t1[:], t2[:], mybir.AluOpType.subtract,
                    )
                    nc.any.tensor_tensor(t3[:], hi, cos_c, mybir.AluOpType.mult)
                    nc.any.tensor_tensor(t4[:], lo, sin_c, mybir.AluOpType.mult)
                    nc.any.tensor_tensor(
                        dst[64:128, bh, s0:s0 + QCH],
                        t3[:], t4[:], mybir.AluOpType.add,
                    )
                else:
                    # v (ct 4,5): cast to bf16, PE-transpose to k-major
                    hl = ct - 4
                    bh = hl * 2 + b
                    vt = vtmp_pool.tile([P, QCH], BF16, tag="vt")
                    nc.vector.tensor_copy(vt[:], ps[:])
                    for blk in range(QCH // P):
                        pt = vtps.tile([P, P], BF16)
                        nc.tensor.transpose(pt[:], vt[:, blk * P:(blk + 1) * P],
                                            ident_sb[:])
                        st = s0 // P + blk
                        nc.vector.tensor_copy(v_sb[:, bh, st, :], pt[:])

        # ---- Phase 3: attention; per-batch AllGather + projection --------
        ag_in = [dram.tile([H_LOC * Dh, S], BF16, name=f"agi{j}")
                 for j in range(B)]
        ag_out = [dram.tile([C, S], BF16, name=f"ago{j}") for j in range(B)]

        def attention(b):
            for qc in range(N_QC):
                n_kt = (QCH // P) * (qc + 1)
                for hl in range(H_LOC):
                    bh = hl * 2 + b
                    po = ops_pool.tile([P, QCH], F32, tag="po")
                    acc = attn_pool.tile([P, QCH], BF16, tag="acc")
                    for kt in range(n_kt):
                        jj = kt - (QCH // P) * qc
                        # diagonal tiles: columns below 128*jj are fully
                        # masked -- skip computing them entirely
                        off = P * jj if jj > 0 else 0
                        pscore = sps_pool.tile([P, QCH], F32, tag="sc")
                        nc.tensor.matmul(
                            pscore[:, off:],
                            lhsT=k_sb[:, bh, kt * P:(kt + 1) * P],
                            rhs=q_sb[:, bh, qc * QCH + off:(qc + 1) * QCH],
                            start=True, stop=True,
                        )
                        pr = probs_pool.tile([P, QCH], BF16, tag="pr")
                        nc.scalar.activation(
                            pr[:, off:], pscore[:, off:],
                            mybir.ActivationFunctionType.Exp,
                            scale=SCALE,
                        )
                        if jj >= 0:
                            nc.vector.tensor_tensor(
                                pr[:, off:], pr[:, off:],
                                masks_sb[:, jj, off:],
                                mybir.AluOpType.mult,
                            )
                        if kt == 0:
                            nc.vector.tensor_copy(acc[:], pr[:])
                        else:
                            nc.vector.tensor_tensor(
                                acc[:, off:], acc[:, off:], pr[:, off:],
                                mybir.AluOpType.add,
                            )
                        nc.tensor.matmul(
                            po[:, off:], lhsT=v_sb[:, bh, kt, :],
                            rhs=pr[:, off:],
                            start=(kt == 0), stop=(kt == n_kt - 1),
                        )
                    pd = mmps.tile([P, QCH], F32, tag="mm")
                    nc.tensor.matmul(
                        pd[:], lhsT=ones_sb[:], rhs=acc[:],
                        start=True, stop=True,
                    )
                    recip = attn_pool.tile([P, QCH], F32, tag="rec")
                    nc.vector.reciprocal(recip[:], pd[:])
                    at = attn_pool.tile([P, QCH], BF16, tag="at")
                    nc.vector.tensor_tensor(
                        at[:], po[:], recip[:], mybir.AluOpType.mult
                    )
                    nc.scalar.dma_start(
                        ag_in[b][hl * Dh:(hl + 1) * Dh,
                                 qc * QCH:(qc + 1) * QCH],
                        at[:],
                    )

        def allgather(b):
            nc.gpsimd.collective_compute(
                "AllGather",
                mybir.AluOpType.bypass,
                replica_groups=[list(range(N_CORES))],
                ins=[ag_in[b][:].opt()],
                outs=[ag_out[b][:].opt()],
            )

        def projection(b):
            for qc in range(N_QC):
                gt = gt_pool.tile([P, N_CC, QCH], BF16, tag="gt")
                nc.scalar.dma_start(
                    gt[:],
                    ag_out[b][:, qc * QCH:(qc + 1) * QCH].rearrange(
                        "(o p) q -> p o q", p=P
                    ),
                )
                for ct in range(CO_LOC // P):
                    ps = mmps.tile([P, QCH], F32, tag="mm")
                    for cc in range(N_CC):
                        nc.tensor.matmul(
                            ps[:],
                            lhsT=wproj_sb[:, cc, ct * P:(ct + 1) * P],
                            rhs=gt[:, cc, :],
                            start=(cc == 0),
                            stop=(cc == N_CC - 1),
                        )
                    ot = outp_pool.tile([P, QCH], F32, tag="ot")
                    nc.vector.tensor_copy(ot[:], ps[:])
                    nc.scalar.dma_start(
                        out_ext[ct * P:(ct + 1) * P,
                                b * S + qc * QCH:b * S + (qc + 1) * QCH],
                        ot[:],
                    )

        attention(0)
        allgather(0)
        attention(1)
        allgather(1)
        projection(0)
        projection(1)

    nc.finalize()
    return nc


_NC_CACHE = None


def _get_nc():
    global _NC_CACHE
    if _NC_CACHE is None:
        _NC_CACHE = build_nc()
    return _NC_CACHE


def make_in_maps(x, Wqkv, Wproj):
    """Shard the full inputs across the 8 cores (host side)."""
    x2 = np.ascontiguousarray(np.asarray(x, dtype=np.float32).reshape(BS, C))
    Wqkv = np.asarray(Wqkv, dtype=np.float32)
    Wproj = np.asarray(Wproj, dtype=np.float32)
    cos_t, sin_t, masks, ident, ones = _host_constants()
    in_maps = []
    for i in range(N_CORES):
        h0 = H_LOC * i
        cols = []
        for part in range(3):  # k, q, v blocks (k first per reference)
            base = part * C + h0 * Dh
            cols.append(Wqkv[:, base:base + H_LOC * Dh])
        wqkv_loc = np.ascontiguousarray(np.concatenate(cols, axis=1))
        wproj_loc = np.ascontiguousarray(Wproj[:, i * CO_LOC:(i + 1) * CO_LOC])
        in_maps.append({
            "x": x2,
            "wqkv": wqkv_loc,
            "wproj": wproj_loc,
            "cos_t": cos_t,
            "sin_t": sin_t,
            "masks": masks,
            "ident": ident,
            "ones": ones,
        })
    return in_maps


def assemble_output(results):
    outT = np.concatenate([results[i]["outT"] for i in range(N_CORES)], axis=0)
    return np.ascontiguousarray(outT.T).reshape(B, S, C).astype(np.float32)


def kernel(x, Wqkv, Wproj):
    nc = _get_nc()
    in_maps = make_in_maps(x, Wqkv, Wproj)
    res = run_bass_kernel_spmd(nc, in_maps, core_ids=list(range(N_CORES)))
    return assemble_output(res.results)
